# revision 7
# baseline (speedup 1.0000x reference)
"""TAGConv-style GNN encoder (degree-normalized edge aggregation + linear +
L2 row-normalize) on 8 Trainium2 NeuronCores.

Strategy (dst-sharded, fully data-parallel — no collectives):
  - Nodes are sharded by destination: core c owns dst rows [c*NPC, (c+1)*NPC).
  - Host-side graph partitioning (integer index metadata only): dedup
    (src,dst) pairs with multiplicity, compute in-degrees, and lay each
    core's edges out into 128-edge tiles grouped by (aligned 64-wide dst
    window, src-chunk). The tile schedule is made identical across cores
    (padded to the per-(window,chunk) max) so one SPMD program serves all 8.
  - Device per core: gpsimd dma_gather (MoE ucode, int16 idxs => gather
    table is split into 4 chunks of 25000 rows) pulls h rows (bf16, 256B
    each) into SBUF tiles [128 edges, 128 feat]; DVE builds a per-tile
    one-hot segment matrix [128 edges, 64 dst-slots] scaled by
    w * rsqrt(deg_src*deg_dst); TensorE matmul G.T @ onehot accumulates
    segment sums straight into PSUM (has_written accumulate semantics let
    dst segments span tiles and src-chunks). Then
    out^T = W1.T @ h^T + W2.T @ agg^T, + bias, L2 row-normalize via a
    ones-matmul partition reduction. Output is written transposed
    [128, NPC_padded]; the host transposes/concatenates shards.
"""
import numpy as np
import ml_dtypes

import concourse.bass as bass
import concourse.tile as tile
from concourse import mybir, bacc
from concourse.bass_utils import run_bass_kernel_spmd

F32 = mybir.dt.float32
BF16 = mybir.dt.bfloat16
I32 = mybir.dt.int32
I16 = mybir.dt.int16


def _patched_drain_and_barrier(self, tick_clock, wait_clock):
    """Tile's kernel-tail Drain carries one sync-wait per outstanding
    semaphore; the walrus build in this container can't encode more than one
    wait on one instruction. Emit each wait as its own wait_ge instead."""
    nc = self.nc
    probe = nc.sync.nop(nofuse=True)
    wait_clock.add_sem_waits(probe.ins, tile.ScopedClock({None: tick_clock.global_clock}))
    si = probe.ins.sync_info
    waits = list(si.on_wait) if si is not None else []
    if len(waits) > 1:
        si.on_wait.clear()
        sem_by_num = {h.num: h for h in self.sems.allocated().values()}
        for w in waits:
            nc.sync.wait_ge(sem_by_num[w.id], w.wait_value)
    nc.sync.drain()
    nc.all_engine_barrier()
    popped = nc._tile_sem_poison_stack.pop()
    assert popped is self._sem_poison
    nc.clear_and_free_semaphores(list(self.sems.allocated().values()))
    nc.all_engine_barrier()


tile.TileContext._drain_and_barrier = _patched_drain_and_barrier

# this walrus build encodes at most this many sync waits on one instruction
MAX_WAITS = 1


def _split_excess_waits(nc, max_waits=MAX_WAITS):
    """Hoist sync waits beyond the per-instruction ISA budget onto NoOps
    inserted just before the instruction (same engine queue, so ordering
    semantics are identical). Must run AFTER Bacc.compile (its nop-fusion
    passes would re-merge the waits)."""
    for f in nc.m.functions:
        for b in f.blocks:
            ins_list = b.instructions
            out_list = []
            changed = False
            for ins in ins_list:
                si = ins.sync_info
                waits = list(si.on_wait) if si is not None else []
                if len(waits) > max_waits:
                    excess, keep = waits[:-max_waits], waits[-max_waits:]
                    for j in range(0, len(excess), max_waits):
                        nop = mybir.InstNoOp(
                            name=nc.get_next_instruction_name(), ins=[], outs=[])
                        nop.engine = ins.engine
                        nop.sync_info = mybir.SyncInfo(
                            on_wait=excess[j:j + max_waits], on_update=[])
                        out_list.append(nop)
                    ins.sync_info = mybir.SyncInfo(
                        on_wait=keep, on_update=list(si.on_update))
                    changed = True
                out_list.append(ins)
            if changed:
                b.instructions = out_list


# Problem constants (hardcoded: harness contract)
N_NODES = 100000
D = 128
HID = 128
CORES = 8

# Kernel tuning
WIN = 256         # dst window width = segment-matmul N
TILE = 128        # edge slots per tile (= matmul K)
BANK = 512        # PSUM bank width in f32 cols
CHUNK_WINS = 6    # windows per PSUM chunk (6*256 = 1536 cols = 3 banks)
GX = 6            # gather tiles per dma_gather instruction (1024 idxs = SWDGE ring cap)
SCH = 4           # src chunks (int16 gather indices => table <= 32767 rows)


def _preprocess(src, dst, n_nodes, npc, cores):
    """Host-side graph partitioning (integer index metadata only)."""
    assert n_nodes % SCH == 0
    cn = n_nodes // SCH
    assert cn < 32768, "src-chunk must fit int16 gather indices"
    src = np.asarray(src).astype(np.int64)
    dst = np.asarray(dst).astype(np.int64)
    deg = np.bincount(dst, minlength=n_nodes)

    # Dedup (dst, src) pairs with multiplicity; result sorted by (dst, src).
    key = dst * n_nodes + src
    ukey, wmul = np.unique(key, return_counts=True)
    udst = ukey // n_nodes
    usrc = ukey % n_nodes

    core_of = udst // npc
    core_bounds = np.searchsorted(core_of, np.arange(cores + 1))
    ldst = udst - core_of * npc
    win = ldst // WIN
    kch = usrc // cn
    n_wins = (npc + WIN - 1) // WIN
    n_codes = n_wins * SCH
    code = win * SCH + kch

    # edges per (core, window, chunk); uniform tiles-per-(w,k) schedule
    cnt = np.zeros((cores, n_codes), np.int64)
    for c in range(cores):
        s, e = core_bounds[c], core_bounds[c + 1]
        cnt[c] = np.bincount(code[s:e], minlength=n_codes)
    tiles_wk = (-(-cnt.max(axis=0) // TILE)).reshape(n_wins, SCH)
    empty = tiles_wk.sum(axis=1) == 0
    tiles_wk[empty, 0] = 1  # every window writes its PSUM cols at least once

    # program tile order: psum-chunk major, then src-chunk, then window
    order = []  # (w, k) per tile
    for p0 in range(0, n_wins, CHUNK_WINS):
        p1 = min(n_wins, p0 + CHUNK_WINS)
        for k in range(SCH):
            for w in range(p0, p1):
                order.extend([(w, k)] * int(tiles_wk[w, k]))
    n_tiles = len(order)
    wk = np.array(order, np.int64)
    win_of_tile = wk[:, 0]
    k_of_tile = wk[:, 1]
    # first slot of each (w,k) region (regions are contiguous in tile order)
    slot_base = np.full(n_codes, -1, np.int64)
    t_acc = 0
    for (w, k) in order:
        if slot_base[w * SCH + k] < 0:
            slot_base[w * SCH + k] = t_acc * TILE
        t_acc += 1
    # recompute properly: slot base = 128 * first tile index of the region
    slot_base = np.full(n_codes, -1, np.int64)
    for t, (w, k) in enumerate(order):
        c_ = w * SCH + k
        if slot_base[c_] < 0:
            slot_base[c_] = t * TILE

    n_slots = n_tiles * TILE

    # gather instruction groups: consecutive tiles of one (w,k) region,
    # <= GX tiles each (pads are region-tail, so per-instruction negative
    # idx tails are legal and num_idxs_reg can skip their descriptors)
    groups = []  # (k, t_start, t_end, region_first_tile, region_code)
    t = 0
    for p0 in range(0, n_wins, CHUNK_WINS):
        p1 = min(n_wins, p0 + CHUNK_WINS)
        for k in range(SCH):
            for w in range(p0, p1):
                nt_r = int(tiles_wk[w, k])
                r0 = t
                for a in range(0, nt_r, GX):
                    b = min(nt_r, a + GX)
                    groups.append((k, r0 + a, r0 + b, r0, w * SCH + k))
                t += nt_r
    assert t == n_tiles

    per_core = []
    for c in range(cores):
        s, e = core_bounds[c], core_bounds[c + 1]
        m = e - s
        # group by (w,k), ascending src within the group (gather locality)
        o = np.lexsort((usrc[s:e], kch[s:e], win[s:e]))
        codes_s = code[s:e][o]
        gstart = np.searchsorted(codes_s, np.arange(n_codes))
        rank = np.arange(m) - gstart[codes_s]
        slot = slot_base[codes_s] + rank

        gidx = np.zeros(n_slots, np.int16)    # pads: row 0 of the chunk (scale 0)
        wm = np.zeros(n_slots, np.float32)
        pclip = np.ones(n_slots, np.float32)
        offs = np.zeros(n_slots, np.float32)

        us, ud, wmc = usrc[s:e][o], udst[s:e][o], wmul[s:e][o]
        gidx[slot] = (us - kch[s:e][o] * cn).astype(np.int16)
        wm[slot] = wmc.astype(np.float32)
        sd = np.maximum(deg[us], 1)
        dd = np.maximum(deg[ud], 1)
        pclip[slot] = (sd * dd).astype(np.float32)
        offs[slot] = (ldst[s:e][o] - win[s:e][o] * WIN).astype(np.float32)

        # [n_slots] -> [128, n_tiles]: slot j of tile t at [j, t]
        def t_(a):
            return np.ascontiguousarray(a.reshape(n_tiles, TILE).T)

        # int16 idx wrap for dma_gather: within-instruction idx i at
        # [i % 16, i // 16], replicated across the 8 16-partition groups.
        # Instruction = run of whole tiles, so per-tile 8-col blocks suffice.
        a = gidx.reshape(n_tiles, 8, 16)          # [t, i//16, i%16]
        wrapped = np.transpose(a, (2, 0, 1)).reshape(16, n_tiles * 8)
        gidx16 = np.ascontiguousarray(np.tile(wrapped, (8, 1)))  # [128, 8*ET]

        # per-instruction real-slot counts for num_idxs_reg
        counts = np.zeros(len(groups), np.int32)
        creg = cnt[c]  # real edges per (w,k) code
        for gi, (k, ta, tb, r0, code_) in enumerate(groups):
            real = int(creg[code_])
            counts[gi] = max(0, min((tb - ta) * TILE, real - (ta - r0) * TILE))
        per_core.append(dict(gidx16=gidx16, wm=t_(wm), pclip=t_(pclip), offs=t_(offs),
                             counts=counts.reshape(1, -1)))

    return dict(
        groups=groups,
        n_wins=n_wins,
        n_tiles=n_tiles,
        win_of_tile=win_of_tile,
        k_of_tile=k_of_tile,
        per_core=per_core,
    )


def _build_program(sched, n_nodes, npc, split_waits=True):
    """Build the single SPMD Bass/Tile program (identical for all cores)."""
    n_wins = sched["n_wins"]
    n_tiles = sched["n_tiles"]
    win_of_tile = sched["win_of_tile"]
    k_of_tile = sched["k_of_tile"]
    cn = n_nodes // SCH
    padn = n_wins * WIN            # padded local dst count (cols of out^T)
    n_chunks = -(-n_wins // CHUNK_WINS)

    nc = bacc.Bacc("TRN2", target_bir_lowering=False)
    hb = nc.declare_dram_parameter("hb", [n_nodes, D], BF16, isOutput=False)
    hself = nc.declare_dram_parameter("hself", [padn, D], BF16, isOutput=False)
    gidx_p = nc.declare_dram_parameter("gidx16", [TILE, 8 * n_tiles], I16, isOutput=False)
    wm_p = nc.declare_dram_parameter("wm", [TILE, n_tiles], F32, isOutput=False)
    pclip_p = nc.declare_dram_parameter("pclip", [TILE, n_tiles], F32, isOutput=False)
    offs_p = nc.declare_dram_parameter("offs", [TILE, n_tiles], F32, isOutput=False)
    wt_p = nc.declare_dram_parameter("wt", [2 * D, HID], BF16, isOutput=False)
    bias_p = nc.declare_dram_parameter("bias_c", [HID, 1], F32, isOutput=False)
    ident_p = nc.declare_dram_parameter("ident", [128, 128], BF16, isOutput=False)
    n_groups = len(sched["groups"])
    cnts_p = nc.declare_dram_parameter("counts", [1, n_groups], I32, isOutput=False)
    out_p = nc.declare_dram_parameter("out", [HID, padn], F32, isOutput=True)

    with tile.TileContext(nc) as tc:
        with (
            tc.tile_pool(name="const", bufs=1) as const,
            tc.tile_pool(name="g", bufs=5) as gpool,
            tc.tile_pool(name="oh", bufs=5) as ohpool,
            tc.tile_pool(name="hr", bufs=2) as hrpool,
            tc.tile_pool(name="slab", bufs=2) as slab,
            tc.tile_pool(name="y", bufs=6) as ypool,
            tc.tile_pool(name="aggps", bufs=1, space="PSUM") as agg_ps,
            tc.tile_pool(name="scrps", bufs=4, space="PSUM") as scr_ps,
        ):
            # ---- constants / metadata ----
            gidx_sb = const.tile([TILE, 8 * n_tiles], I16)
            nc.sync.dma_start(gidx_sb[:], gidx_p[:])
            wm_sb = const.tile([TILE, n_tiles], F32)
            nc.sync.dma_start(wm_sb[:], wm_p[:])
            pclip_sb = const.tile([TILE, n_tiles], F32)
            nc.sync.dma_start(pclip_sb[:], pclip_p[:])
            offs_sb = const.tile([TILE, n_tiles], F32)
            nc.sync.dma_start(offs_sb[:], offs_p[:])

            w1_sb = const.tile([D, HID], BF16)
            nc.sync.dma_start(w1_sb[:], wt_p[0:D, :])
            w2_sb = const.tile([D, HID], BF16)
            nc.sync.dma_start(w2_sb[:], wt_p[D:2 * D, :])
            bias_sb = const.tile([HID, 1], F32)
            nc.sync.dma_start(bias_sb[:], bias_p[:])
            ident_sb = const.tile([128, 128], BF16)
            nc.sync.dma_start(ident_sb[:], ident_p[:])
            cnts_sb = const.tile([1, n_groups], I32)
            nc.sync.dma_start(cnts_sb[:], cnts_p[:])
            ones_sb = const.tile([128, 128], F32)
            nc.vector.memset(ones_sb[:], 1.0)
            iota_i = const.tile([128, WIN], I32)
            nc.gpsimd.iota(iota_i[:], pattern=[[1, WIN]], base=0, channel_multiplier=0)
            iota_b = const.tile([128, WIN], BF16)
            nc.vector.tensor_copy(iota_b[:], iota_i[:])

            # per-slot scale = wm * rsqrt(pclip), pclip = clip(deg_s)*clip(deg_d)
            scale_f = const.tile([TILE, n_tiles], F32)
            nc.vector.reciprocal(scale_f[:], pclip_sb[:])
            nc.scalar.sqrt(scale_f[:], scale_f[:])
            scale_b = const.tile([TILE, n_tiles], BF16)
            nc.vector.tensor_tensor(out=scale_b[:], in0=scale_f[:], in1=wm_sb[:],
                                    op=mybir.AluOpType.mult)
            offs_b = const.tile([TILE, n_tiles], BF16)
            nc.vector.tensor_copy(offs_b[:], offs_sb[:])

            # shared num_idxs registers for dma_gather (one per distinct size)
            ni_regs = {}

            def ni_reg(n):
                if n not in ni_regs:
                    r = nc.gpsimd.alloc_register()
                    nc.gpsimd.reg_mov(r, n)
                    ni_regs[n] = r
                return ni_regs[n]

            group_by_start = {g[1]: (gi, g) for gi, g in enumerate(sched["groups"])}

            # cumulative tile index at each window boundary is not enough now;
            # precompute per-chunk tile ranges from the global order
            tile_of_chunk = [[] for _ in range(n_chunks)]
            for t in range(n_tiles):
                tile_of_chunk[int(win_of_tile[t]) // CHUNK_WINS].append(t)

            # ---- main loop over dst chunks ----
            for ch in range(n_chunks):
                w0 = ch * CHUNK_WINS
                w1 = min(n_wins, w0 + CHUNK_WINS)
                cw = (w1 - w0) * WIN
                col0 = w0 * WIN
                tlist = tile_of_chunk[ch]
                assert tlist == list(range(tlist[0], tlist[-1] + 1))
                t0c, t1c = tlist[0], tlist[-1] + 1

                # first/last program-order touch per psum bank in this chunk
                bank_of = [(int(win_of_tile[t]) - w0) * WIN // BANK for t in tlist]
                first_of_bank, last_of_bank = {}, {}
                for t, bk in zip(tlist, bank_of):
                    first_of_bank.setdefault(bk, t)
                    last_of_bank[bk] = t

                pagg = agg_ps.tile([128, CHUNK_WINS * WIN], F32, tag="pagg")

                # gather groups: region-aligned runs, up to GX tiles
                g0 = t0c
                while g0 < t1c:
                    gi, (k, ta, gend, r0, code_) = group_by_start[g0]
                    assert ta == g0
                    gt = gend - g0
                    G = gpool.tile([128, GX, D], BF16, tag="G")
                    nc.gpsimd.dma_gather(
                        out_ap=G[:, :gt, :],
                        in_ap=hb[k * cn:(k + 1) * cn, :],
                        idxs_ap=gidx_sb[:, 8 * g0:8 * gend],
                        num_idxs=TILE * gt,
                        num_idxs_reg=ni_reg(TILE * gt),
                        elem_size=D,
                    )
                    oh = ohpool.tile([128, GX, WIN], BF16, tag="oh")
                    off_bc = offs_b[:, g0:gend].unsqueeze(2).broadcast_to([128, gt, WIN])
                    iota_bc = iota_b[:].unsqueeze(1).broadcast_to([128, gt, WIN])
                    nc.vector.tensor_tensor(out=oh[:, :gt, :], in0=off_bc, in1=iota_bc,
                                            op=mybir.AluOpType.is_equal)
                    sc_bc = scale_b[:, g0:gend].unsqueeze(2).broadcast_to([128, gt, WIN])
                    nc.vector.tensor_tensor(out=oh[:, :gt, :], in0=oh[:, :gt, :],
                                            in1=sc_bc, op=mybir.AluOpType.mult)
                    for x in range(gt):
                        t = g0 + x
                        col = (int(win_of_tile[t]) - w0) * WIN
                        bk = bank_of[t - t0c]
                        nc.tensor.matmul(
                            pagg[:, col:col + WIN],
                            lhsT=G[:, x, :],
                            rhs=oh[:, x, :],
                            start=(first_of_bank[bk] == t),
                            stop=(last_of_bank[bk] == t),
                            skip_group_check=True,
                        )
                    g0 = gend

                # evacuate agg chunk (cast to bf16; norms folded into scale)
                aggT = slab.tile([128, CHUNK_WINS * WIN], BF16, tag="aggT")
                nc.vector.tensor_copy(aggT[:, :cw], pagg[:, :cw])

                # h^T slab for this chunk's dst rows via PE transpose
                nh = cw // 128
                hr = hrpool.tile([128, CHUNK_WINS * WIN // 128, D], BF16, tag="hr")
                nc.sync.dma_start(
                    hr[:, :nh, :],
                    hself[col0:col0 + cw, :].rearrange("(x p) f -> p x f", p=128),
                )
                hT = slab.tile([128, CHUNK_WINS * WIN], BF16, tag="hT")
                for xt in range(nh):
                    pt = scr_ps.tile([128, 128], BF16, tag="scr")
                    nc.tensor.transpose(pt[:], hr[:, xt, :], ident_sb[:])
                    nc.vector.tensor_copy(hT[:, xt * 128:(xt + 1) * 128], pt[:])

                # out^T = W1.T @ h^T + W2.T @ agg^T ; + bias; L2 normalize; store
                for bs in range(0, cw, BANK):
                    bw = min(BANK, cw - bs)
                    po = scr_ps.tile([128, BANK], F32, tag="scr")
                    nc.tensor.matmul(po[:, :bw], lhsT=w1_sb[:], rhs=hT[:, bs:bs + bw],
                                     start=True, stop=False)
                    nc.tensor.matmul(po[:, :bw], lhsT=w2_sb[:], rhs=aggT[:, bs:bs + bw],
                                     start=False, stop=True)
                    y = ypool.tile([128, BANK], F32, tag="y")
                    nc.scalar.activation(y[:, :bw], po[:, :bw],
                                         mybir.ActivationFunctionType.Identity,
                                         bias=bias_sb[:])
                    z = ypool.tile([128, BANK], F32, tag="z")
                    nc.scalar.square(z[:, :bw], y[:, :bw])
                    pr = scr_ps.tile([128, BANK], F32, tag="scr")
                    nc.tensor.matmul(pr[:, :bw], lhsT=ones_sb[:], rhs=z[:, :bw],
                                     start=True, stop=True)
                    rs = ypool.tile([128, BANK], F32, tag="rs")
                    nc.vector.reciprocal(rs[:, :bw], pr[:, :bw])
                    nc.scalar.sqrt(rs[:, :bw], rs[:, :bw])
                    of = ypool.tile([128, BANK], F32, tag="of")
                    nc.vector.tensor_tensor(out=of[:, :bw], in0=y[:, :bw],
                                            in1=rs[:, :bw], op=mybir.AluOpType.mult)
                    nc.sync.dma_start(out_p[:, col0 + bs:col0 + bs + bw], of[:, :bw])

    nc.finalize()
    if split_waits:
        _split_excess_waits(nc)
    return nc


def _run(h, weight, bias, src, dst, n_nodes, npc, cores, trace=False):
    sched = _preprocess(src, dst, n_nodes, npc, cores)
    nc = _build_program(sched, n_nodes, npc)

    padn = sched["n_wins"] * WIN
    h = np.asarray(h, dtype=np.float32)
    hb = h.astype(ml_dtypes.bfloat16)
    wt = np.asarray(weight, dtype=np.float32).astype(ml_dtypes.bfloat16)
    bias_c = np.ascontiguousarray(np.asarray(bias, dtype=np.float32).reshape(HID, 1))
    ident = np.eye(128, dtype=np.float32).astype(ml_dtypes.bfloat16)

    in_maps = []
    for c in range(cores):
        pc = sched["per_core"][c]
        hself = np.zeros((padn, D), dtype=ml_dtypes.bfloat16)
        hself[:npc] = hb[c * npc:(c + 1) * npc]
        in_maps.append(dict(
            hb=hb, hself=hself,
            gidx16=pc["gidx16"], wm=pc["wm"], pclip=pc["pclip"], offs=pc["offs"],
            counts=pc["counts"], wt=wt, bias_c=bias_c, ident=ident,
        ))

    res = run_bass_kernel_spmd(nc, in_maps, core_ids=list(range(cores)), trace=trace)
    out = np.empty((cores * npc, HID), dtype=np.float32)
    for c in range(cores):
        out[c * npc:(c + 1) * npc] = res.results[c]["out"][:, :npc].T
    return out, res


def kernel(h, weight, bias, src, dst):
    out, _ = _run(h, weight, bias, src, dst, N_NODES, N_NODES // CORES, CORES)
    return out



# revision 8
# speedup vs baseline: 1.1957x; 1.1957x over previous
"""TAGConv-style GNN encoder (degree-normalized edge aggregation + linear +
L2 row-normalize) on 8 Trainium2 NeuronCores.

Strategy (dst-sharded, fully data-parallel — no collectives):
  - Nodes sharded by destination: core c owns dst rows [c*NPC, (c+1)*NPC).
  - Host-side metadata: edges (with multiplicity — no dedup) are laid out
    into 128-edge tiles grouped by (256-wide dst window, src-chunk of 25000
    rows). The tile schedule is shared across cores (padded to the
    per-region max) so one SPMD program serves all 8.
  - Gather: the h table is pre-scaled by rsqrt(deg_src) on host (bf16).
    Per region, one big gpsimd dma_gather in PREPARE_ONLY mode writes SWDGE
    descriptors; trigger_dma fires them. 4 SWDGE queues + a 4096-descriptor
    ring let descriptor generation overlap the DMA transfers, so the DMA
    engines (not the gpsimd ucode) are the limiter.
  - Scatter: DVE tensor_scalar builds per-tile one-hot segment matrices
    oh[slot, dstoff] = (iota == offs[slot]) * rsqrt(deg_dst[slot]) in one
    4x-mode instruction per tile; TensorE matmul G.T @ oh accumulates
    segment sums in PSUM across tiles (has_written semantics).
  - Tail: out^T = W1.T @ h^T + W2.T @ agg^T (+bias), L2-normalize columns
    via ones-matmul partition reduction + scalar-engine Rsqrt. h^T comes
    pre-transposed from the host. Output is written transposed
    [128, NPC_padded]; the host transposes/concatenates shards.
"""
import numpy as np
import ml_dtypes

import concourse.bass as bass
import concourse.tile as tile
from concourse import mybir, bacc
from concourse.bass_utils import run_bass_kernel_spmd

F32 = mybir.dt.float32
BF16 = mybir.dt.bfloat16
I32 = mybir.dt.int32
I16 = mybir.dt.int16


def _patched_drain_and_barrier(self, tick_clock, wait_clock):
    """Tile's kernel-tail Drain carries one sync-wait per outstanding
    semaphore; the walrus build in this container can't encode more than one
    wait on one instruction. Emit each wait as its own wait_ge instead."""
    nc = self.nc
    probe = nc.sync.nop(nofuse=True)
    wait_clock.add_sem_waits(probe.ins, tile.ScopedClock({None: tick_clock.global_clock}))
    si = probe.ins.sync_info
    waits = list(si.on_wait) if si is not None else []
    if len(waits) > 1:
        si.on_wait.clear()
        sem_by_num = {h.num: h for h in self.sems.allocated().values()}
        for w in waits:
            nc.sync.wait_ge(sem_by_num[w.id], w.wait_value)
    nc.sync.drain()
    nc.all_engine_barrier()
    popped = nc._tile_sem_poison_stack.pop()
    assert popped is self._sem_poison
    nc.clear_and_free_semaphores(list(self.sems.allocated().values()))
    nc.all_engine_barrier()


tile.TileContext._drain_and_barrier = _patched_drain_and_barrier

# this walrus build encodes at most this many sync waits on one instruction
MAX_WAITS = 1


def _split_excess_waits(nc, max_waits=MAX_WAITS):
    """Hoist sync waits beyond the per-instruction ISA budget onto NoOps
    inserted just before the instruction (same engine queue, so ordering
    semantics are identical). Must run AFTER Bacc.compile (its nop-fusion
    passes would re-merge the waits)."""
    for f in nc.m.functions:
        for b in f.blocks:
            ins_list = b.instructions
            out_list = []
            changed = False
            for ins in ins_list:
                si = ins.sync_info
                waits = list(si.on_wait) if si is not None else []
                if len(waits) > max_waits:
                    excess, keep = waits[:-max_waits], waits[-max_waits:]
                    for j in range(0, len(excess), max_waits):
                        nop = mybir.InstNoOp(
                            name=nc.get_next_instruction_name(), ins=[], outs=[])
                        nop.engine = ins.engine
                        nop.sync_info = mybir.SyncInfo(
                            on_wait=excess[j:j + max_waits], on_update=[])
                        out_list.append(nop)
                    ins.sync_info = mybir.SyncInfo(
                        on_wait=keep, on_update=list(si.on_update))
                    changed = True
                out_list.append(ins)
            if changed:
                b.instructions = out_list


# Problem constants (hardcoded: harness contract)
N_NODES = 100000
D = 128
HID = 128
CORES = 8

# Kernel tuning
WIN = 256         # dst window width = segment-matmul N
TILE = 128        # edge slots per tile (= matmul K)
BANK = 512        # PSUM bank width in f32 cols
CHUNK_WINS = 6    # windows per PSUM chunk (6*256 = 1536 cols = 3 banks)
GXMAX = 8         # max tiles per dma_gather
SCH = 4           # src chunks (int16 gather indices => table <= 32767 rows)
NQ = 1            # SWDGE queues
SCRATCH = 16384   # dynamic DMA scratch bytes/partition
PREP = False       # prepare_only + trigger_dma (False: blocking dma_gather)
OFFS_PAD = 512.0  # one-hot match value for pad slots (never matches iota)


def _preprocess(src, dst, n_nodes, npc, cores):
    """Host-side edge partitioning (integer index metadata only)."""
    assert n_nodes % SCH == 0
    cn = n_nodes // SCH
    assert cn < 32768, "src-chunk must fit int16 gather indices"
    src = np.asarray(src).astype(np.int64)
    dst = np.asarray(dst).astype(np.int64)
    E = len(src)
    deg = np.bincount(dst, minlength=n_nodes)
    nrm = 1.0 / np.sqrt(np.maximum(deg, 1).astype(np.float64))

    core_of = dst // npc
    ldst = dst - core_of * npc
    win = ldst // WIN
    kch = src // cn
    n_wins = (npc + WIN - 1) // WIN
    n_regions = n_wins * SCH
    region = win * SCH + kch  # (w, k) region id within a core

    # program emission order of regions: psum-chunk major, then k, then w
    order_regions = []
    for p0 in range(0, n_wins, CHUNK_WINS):
        p1 = min(n_wins, p0 + CHUNK_WINS)
        for k in range(SCH):
            for w in range(p0, p1):
                order_regions.append(w * SCH + k)
    order_regions = np.array(order_regions, np.int64)
    region_pos = np.empty(n_regions, np.int64)
    region_pos[order_regions] = np.arange(n_regions)

    # per-core edge counts per region; shared tile schedule = per-region max
    cnt = np.zeros((cores, n_regions), np.int64)
    for c in range(cores):
        m = core_of == c
        cnt[c] = np.bincount(region[m], minlength=n_regions)
    tiles_r = -(-cnt.max(axis=0) // TILE)  # ceil
    # every window must write its PSUM cols at least once
    win_tiles = tiles_r.reshape(n_wins, SCH).sum(axis=1)
    for w in np.nonzero(win_tiles == 0)[0]:
        tiles_r[w * SCH] = 1

    T_sorted = tiles_r[order_regions]
    tile_base_sorted = np.zeros(n_regions + 1, np.int64)
    np.cumsum(T_sorted, out=tile_base_sorted[1:])
    n_tiles = int(tile_base_sorted[-1])
    n_slots = n_tiles * TILE
    slot_base_pos = tile_base_sorted[:-1] * TILE  # by emission pos

    # slot assignment: sort edges by (core, region emission pos, src)
    rpos = region_pos[region]
    glob = np.lexsort((src, rpos, core_of))
    cs, rs = core_of[glob], rpos[glob]
    runkey = cs * n_regions + rs
    starts = np.searchsorted(runkey, np.arange(cores * n_regions))
    rank = np.arange(E) - starts[runkey]
    slot = slot_base_pos[rs] + rank

    core_bounds = np.searchsorted(cs, np.arange(cores + 1))

    per_core = []
    for c in range(cores):
        s, e = core_bounds[c], core_bounds[c + 1]
        sl = slot[s:e]
        g = glob[s:e]
        gidx = np.zeros(n_slots, np.int16)        # pads gather row 0 of chunk
        offs = np.full(n_slots, OFFS_PAD, np.float32)
        nrmd = np.zeros(n_slots, np.float32)
        gidx[sl] = (src[g] - kch[g] * cn).astype(np.int16)
        offs[sl] = (ldst[g] - win[g] * WIN).astype(np.float32)
        nrmd[sl] = nrm[dst[g]].astype(np.float32)

        # [n_slots] -> [128, n_tiles]: slot j of tile t at [j, t]
        def t_(a, dt):
            return np.ascontiguousarray(a.reshape(n_tiles, TILE).T.astype(dt))

        # int16 idx wrap for dma_gather: within-instruction idx i at
        # [i % 16, i // 16], replicated across the 8 16-partition groups.
        # Instruction = run of whole tiles, so per-tile 8-col blocks suffice.
        a = gidx.reshape(n_tiles, 8, 16)          # [t, i//16, i%16]
        wrapped = np.transpose(a, (2, 0, 1)).reshape(16, n_tiles * 8)
        gidx16 = np.ascontiguousarray(np.tile(wrapped, (8, 1)))  # [128, 8*NT]

        per_core.append(dict(
            gidx16=gidx16,
            offs=t_(offs, np.float32),
            nrmd=t_(nrmd, np.float32),
        ))

    # gather batches: consecutive same-k regions packed up to GXMAX tiles,
    # never crossing a psum-chunk boundary
    groups = []  # (k, t_start, t_end)
    n_chunks = -(-n_wins // CHUNK_WINS)
    pos = 0
    t_acc = 0
    for p0 in range(0, n_wins, CHUNK_WINS):
        p1 = min(n_wins, p0 + CHUNK_WINS)
        for k in range(SCH):
            run = []  # tile counts of this (chunk, k) run of regions
            for w in range(p0, p1):
                run.append(int(tiles_r[w * SCH + k]))
            total = sum(run)
            # split [t_acc, t_acc+total) into <=GXMAX batches
            a = 0
            while a < total:
                b = min(total, a + GXMAX)
                groups.append((k, t_acc + a, t_acc + b))
                a = b
            t_acc += total
            pos += len(run)
    assert t_acc == n_tiles

    # window of each tile (for matmul column/bank mapping)
    win_of_tile = np.empty(n_tiles, np.int64)
    t = 0
    for p0 in range(0, n_wins, CHUNK_WINS):
        p1 = min(n_wins, p0 + CHUNK_WINS)
        for k in range(SCH):
            for w in range(p0, p1):
                nt = int(tiles_r[w * SCH + k])
                win_of_tile[t:t + nt] = w
                t += nt

    return dict(
        groups=groups,
        n_wins=n_wins,
        n_tiles=n_tiles,
        win_of_tile=win_of_tile,
        nrm=nrm,
        per_core=per_core,
    )


def _build_program(sched, n_nodes, npc, split_waits=True):
    """Build the single SPMD Bass/Tile program (identical for all cores)."""
    n_wins = sched["n_wins"]
    n_tiles = sched["n_tiles"]
    win_of_tile = sched["win_of_tile"]
    groups = sched["groups"]
    cn = n_nodes // SCH
    padn = n_wins * WIN            # padded local dst count (cols of out^T)
    n_chunks = -(-n_wins // CHUNK_WINS)

    nc = bacc.Bacc("TRN2", target_bir_lowering=False,
                   num_swdge_queues=NQ, dynamic_dma_scratch_size=SCRATCH)
    hb = nc.declare_dram_parameter("hb", [n_nodes, D], BF16, isOutput=False)
    hTd = nc.declare_dram_parameter("hTd", [D, padn], BF16, isOutput=False)
    gidx_p = nc.declare_dram_parameter("gidx16", [TILE, 8 * n_tiles], I16, isOutput=False)
    offs_p = nc.declare_dram_parameter("offs", [TILE, n_tiles], F32, isOutput=False)
    nrmd_p = nc.declare_dram_parameter("nrmd", [TILE, n_tiles], F32, isOutput=False)
    wt_p = nc.declare_dram_parameter("wt", [2 * D, HID], BF16, isOutput=False)
    bias_p = nc.declare_dram_parameter("bias_c", [HID, 1], F32, isOutput=False)
    out_p = nc.declare_dram_parameter("out", [HID, padn], F32, isOutput=True)

    # tiles grouped per psum chunk
    chunk_of_tile = win_of_tile // CHUNK_WINS

    with tile.TileContext(nc) as tc:
        with (
            tc.tile_pool(name="const", bufs=1) as const,
            tc.tile_pool(name="g", bufs=3) as gpool,
            tc.tile_pool(name="oh", bufs=3) as ohpool,
            tc.tile_pool(name="ht", bufs=2) as htpool,
            tc.tile_pool(name="at", bufs=2) as atpool,
            tc.tile_pool(name="y", bufs=6) as ypool,
            tc.tile_pool(name="aggps", bufs=1, space="PSUM") as agg_ps,
            tc.tile_pool(name="scrps", bufs=2, space="PSUM") as scr_ps,
        ):
            # ---- constants / metadata ----
            gidx_sb = const.tile([TILE, 8 * n_tiles], I16)
            nc.sync.dma_start(gidx_sb[:], gidx_p[:])
            offs_sb = const.tile([TILE, n_tiles], F32)
            nc.sync.dma_start(offs_sb[:], offs_p[:])
            nrmd_sb = const.tile([TILE, n_tiles], F32)
            nc.sync.dma_start(nrmd_sb[:], nrmd_p[:])

            w1_sb = const.tile([D, HID], BF16)
            nc.sync.dma_start(w1_sb[:], wt_p[0:D, :])
            w2_sb = const.tile([D, HID], BF16)
            nc.sync.dma_start(w2_sb[:], wt_p[D:2 * D, :])
            bias_sb = const.tile([HID, 1], F32)
            nc.sync.dma_start(bias_sb[:], bias_p[:])
            ones_sb = const.tile([128, 128], BF16)
            nc.vector.memset(ones_sb[:], 1.0)
            iota_i = const.tile([128, WIN], I32)
            nc.gpsimd.iota(iota_i[:], pattern=[[1, WIN]], base=0, channel_multiplier=0)
            iota_b = const.tile([128, WIN], BF16)
            nc.vector.tensor_copy(iota_b[:], iota_i[:])

            # shared num_idxs registers for dma_gather (one per distinct size)
            ni_regs = {}

            def ni_reg(n):
                if n not in ni_regs:
                    r = nc.gpsimd.alloc_register()
                    nc.gpsimd.reg_mov(r, n)
                    ni_regs[n] = r
                return ni_regs[n]

            dma_sems = [nc.alloc_semaphore(f"gdma{q}") for q in range(NQ)]

            # per-chunk batch lists
            batches_of_chunk = [[] for _ in range(n_chunks)]
            for gi, (k, ta, tb) in enumerate(groups):
                ch = int(chunk_of_tile[ta])
                assert int(chunk_of_tile[tb - 1]) == ch
                batches_of_chunk[ch].append((k, ta, tb))

            qrr = 0  # round-robin queue cursor

            # ---- main loop over dst chunks ----
            for ch in range(n_chunks):
                w0 = ch * CHUNK_WINS
                w1 = min(n_wins, w0 + CHUNK_WINS)
                cw = (w1 - w0) * WIN
                col0 = w0 * WIN

                tlist = [t for (k, ta, tb) in batches_of_chunk[ch]
                         for t in range(ta, tb)]
                # first/last program-order touch per psum bank in this chunk
                first_of_bank, last_of_bank = {}, {}
                for t in tlist:
                    bk = (int(win_of_tile[t]) - w0) * WIN // BANK
                    first_of_bank.setdefault(bk, t)
                    last_of_bank[bk] = t

                pagg = agg_ps.tile([128, CHUNK_WINS * WIN], F32, tag="pagg")

                for (k, ta, tb) in batches_of_chunk[ch]:
                    gt = tb - ta
                    G = gpool.tile([128, GXMAX, D], BF16, tag="G")
                    q = qrr % NQ
                    qrr += 1
                    if PREP:
                        nc.gpsimd.dma_gather(
                            out_ap=G[:, :gt, :],
                            in_ap=hb[k * cn:(k + 1) * cn, :],
                            idxs_ap=gidx_sb[:, 8 * ta:8 * tb],
                            num_idxs=TILE * gt,
                            num_idxs_reg=ni_reg(TILE * gt),
                            elem_size=D,
                            prepare_only=True,
                            sem=dma_sems[q],
                            queue_num=q,
                        )
                        nc.gpsimd.trigger_dma(count=None, queue_num=q)
                    else:
                        nc.gpsimd.dma_gather(
                            out_ap=G[:, :gt, :],
                            in_ap=hb[k * cn:(k + 1) * cn, :],
                            idxs_ap=gidx_sb[:, 8 * ta:8 * tb],
                            num_idxs=TILE * gt,
                            num_idxs_reg=ni_reg(TILE * gt),
                            elem_size=D,
                            queue_num=q,
                        )

                    oh = ohpool.tile([128, GXMAX, WIN], BF16, tag="oh")
                    for x in range(gt):
                        t = ta + x
                        nc.vector.tensor_scalar(
                            out=oh[:, x, :],
                            in0=iota_b[:],
                            scalar1=offs_sb[:, t:t + 1],
                            scalar2=nrmd_sb[:, t:t + 1],
                            op0=mybir.AluOpType.is_equal,
                            op1=mybir.AluOpType.mult,
                        )
                    for x in range(gt):
                        t = ta + x
                        col = (int(win_of_tile[t]) - w0) * WIN
                        bk = col // BANK
                        nc.tensor.matmul(
                            pagg[:, col:col + WIN],
                            lhsT=G[:, x, :],
                            rhs=oh[:, x, :],
                            start=(first_of_bank[bk] == t),
                            stop=(last_of_bank[bk] == t),
                            skip_group_check=True,
                        )

                # evacuate agg chunk (cast to bf16; norms folded into oh/hb)
                aggT = atpool.tile([128, CHUNK_WINS * WIN], BF16, tag="aggT")
                nc.vector.tensor_copy(aggT[:, :cw], pagg[:, :cw])

                # h^T slab (host-pretransposed)
                hT = htpool.tile([128, CHUNK_WINS * WIN], BF16, tag="hT")
                nc.sync.dma_start(hT[:, :cw], hTd[:, col0:col0 + cw])

                # out^T = W1.T @ h^T + W2.T @ agg^T ; +bias; L2 normalize
                for bs in range(0, cw, BANK):
                    bw = min(BANK, cw - bs)
                    po = scr_ps.tile([128, BANK], F32, tag="po")
                    nc.tensor.matmul(po[:, :bw], lhsT=w1_sb[:], rhs=hT[:, bs:bs + bw],
                                     start=True, stop=False)
                    nc.tensor.matmul(po[:, :bw], lhsT=w2_sb[:], rhs=aggT[:, bs:bs + bw],
                                     start=False, stop=True)
                    y = ypool.tile([128, BANK], F32, tag="y")
                    nc.scalar.activation(y[:, :bw], po[:, :bw],
                                         mybir.ActivationFunctionType.Identity,
                                         bias=bias_sb[:])
                    z = ypool.tile([128, BANK], BF16, tag="z")
                    nc.scalar.square(z[:, :bw], y[:, :bw])
                    pr = scr_ps.tile([128, BANK], F32, tag="pr")
                    nc.tensor.matmul(pr[:, :bw], lhsT=ones_sb[:], rhs=z[:, :bw],
                                     start=True, stop=True)
                    lg = ypool.tile([128, BANK], F32, tag="lg")
                    nc.scalar.activation(lg[:, :bw], pr[:, :bw],
                                         mybir.ActivationFunctionType.Ln)
                    rs = ypool.tile([128, BANK], F32, tag="rs")
                    nc.scalar.activation(rs[:, :bw], lg[:, :bw],
                                         mybir.ActivationFunctionType.Exp,
                                         scale=-0.5)
                    of = ypool.tile([128, BANK], F32, tag="of")
                    nc.vector.tensor_tensor(out=of[:, :bw], in0=y[:, :bw],
                                            in1=rs[:, :bw], op=mybir.AluOpType.mult)
                    nc.sync.dma_start(out_p[:, col0 + bs:col0 + bs + bw], of[:, :bw])

    nc.finalize()
    if split_waits:
        _split_excess_waits(nc)
    return nc


def _run(h, weight, bias, src, dst, n_nodes, npc, cores, trace=False):
    sched = _preprocess(src, dst, n_nodes, npc, cores)
    nc = _build_program(sched, n_nodes, npc)

    padn = sched["n_wins"] * WIN
    h = np.asarray(h, dtype=np.float32)
    # gather table pre-scaled by rsqrt(deg_src)
    hb = (h * sched["nrm"][:, None].astype(np.float32)).astype(ml_dtypes.bfloat16)
    wt = np.asarray(weight, dtype=np.float32).astype(ml_dtypes.bfloat16)
    bias_c = np.ascontiguousarray(np.asarray(bias, dtype=np.float32).reshape(HID, 1))

    in_maps = []
    for c in range(cores):
        pc = sched["per_core"][c]
        hTd = np.zeros((D, padn), dtype=ml_dtypes.bfloat16)
        hTd[:, :npc] = h[c * npc:(c + 1) * npc].T.astype(ml_dtypes.bfloat16)
        in_maps.append(dict(
            hb=hb, hTd=np.ascontiguousarray(hTd),
            gidx16=pc["gidx16"], offs=pc["offs"], nrmd=pc["nrmd"],
            wt=wt, bias_c=bias_c,
        ))

    res = run_bass_kernel_spmd(nc, in_maps, core_ids=list(range(cores)), trace=trace)
    out = np.empty((cores * npc, HID), dtype=np.float32)
    for c in range(cores):
        out[c * npc:(c + 1) * npc] = res.results[c]["out"][:, :npc].T
    return out, res


def kernel(h, weight, bias, src, dst):
    out, _ = _run(h, weight, bias, src, dst, N_NODES, N_NODES // CORES, CORES)
    return out


# revision 9
# speedup vs baseline: 1.4234x; 1.1905x over previous
"""TAGConv-style GNN encoder (degree-normalized edge aggregation + linear +
L2 row-normalize) on 8 Trainium2 NeuronCores.

Strategy (dst-sharded, fully data-parallel — no collectives):
  - Nodes sharded by destination: core c owns dst rows [c*NPC, (c+1)*NPC).
  - Host-side metadata: edges (with multiplicity — no dedup) are laid out
    into 128-edge tiles grouped by (256-wide dst window, src-chunk of 25000
    rows). The tile schedule is shared across cores (padded to the
    per-region max) so one SPMD program serves all 8.
  - Gather: the h table is pre-scaled by rsqrt(deg_src) on host (bf16).
    Per region, one big gpsimd dma_gather in PREPARE_ONLY mode writes SWDGE
    descriptors; trigger_dma fires them. 4 SWDGE queues + a 4096-descriptor
    ring let descriptor generation overlap the DMA transfers, so the DMA
    engines (not the gpsimd ucode) are the limiter.
  - Scatter: DVE tensor_scalar builds per-tile one-hot segment matrices
    oh[slot, dstoff] = (iota == offs[slot]) * rsqrt(deg_dst[slot]) in one
    4x-mode instruction per tile; TensorE matmul G.T @ oh accumulates
    segment sums in PSUM across tiles (has_written semantics).
  - Tail: out^T = W1.T @ h^T + W2.T @ agg^T (+bias), L2-normalize columns
    via ones-matmul partition reduction + scalar-engine Rsqrt. h^T comes
    pre-transposed from the host. Output is written transposed
    [128, NPC_padded]; the host transposes/concatenates shards.
"""
import numpy as np
import ml_dtypes

import concourse.bass as bass
import concourse.tile as tile
from concourse import mybir, bacc
from concourse.bass_utils import run_bass_kernel_spmd

F32 = mybir.dt.float32
BF16 = mybir.dt.bfloat16
I32 = mybir.dt.int32
I16 = mybir.dt.int16


def _patched_drain_and_barrier(self, tick_clock, wait_clock):
    """Tile's kernel-tail Drain carries one sync-wait per outstanding
    semaphore; the walrus build in this container can't encode more than one
    wait on one instruction. Emit each wait as its own wait_ge instead."""
    nc = self.nc
    probe = nc.sync.nop(nofuse=True)
    wait_clock.add_sem_waits(probe.ins, tile.ScopedClock({None: tick_clock.global_clock}))
    si = probe.ins.sync_info
    waits = list(si.on_wait) if si is not None else []
    if len(waits) > 1:
        si.on_wait.clear()
        sem_by_num = {h.num: h for h in self.sems.allocated().values()}
        for w in waits:
            nc.sync.wait_ge(sem_by_num[w.id], w.wait_value)
    nc.sync.drain()
    nc.all_engine_barrier()
    popped = nc._tile_sem_poison_stack.pop()
    assert popped is self._sem_poison
    nc.clear_and_free_semaphores(list(self.sems.allocated().values()))
    nc.all_engine_barrier()


tile.TileContext._drain_and_barrier = _patched_drain_and_barrier

# this walrus build encodes at most this many sync waits on one instruction
MAX_WAITS = 1


def _split_excess_waits(nc, max_waits=MAX_WAITS):
    """Hoist sync waits beyond the per-instruction ISA budget onto NoOps
    inserted just before the instruction (same engine queue, so ordering
    semantics are identical). Must run AFTER Bacc.compile (its nop-fusion
    passes would re-merge the waits)."""
    for f in nc.m.functions:
        for b in f.blocks:
            ins_list = b.instructions
            out_list = []
            changed = False
            for ins in ins_list:
                si = ins.sync_info
                waits = list(si.on_wait) if si is not None else []
                if len(waits) > max_waits:
                    excess, keep = waits[:-max_waits], waits[-max_waits:]
                    for j in range(0, len(excess), max_waits):
                        nop = mybir.InstNoOp(
                            name=nc.get_next_instruction_name(), ins=[], outs=[])
                        nop.engine = ins.engine
                        nop.sync_info = mybir.SyncInfo(
                            on_wait=excess[j:j + max_waits], on_update=[])
                        out_list.append(nop)
                    ins.sync_info = mybir.SyncInfo(
                        on_wait=keep, on_update=list(si.on_update))
                    changed = True
                out_list.append(ins)
            if changed:
                b.instructions = out_list


# Problem constants (hardcoded: harness contract)
N_NODES = 100000
D = 128
HID = 128
CORES = 8

# Kernel tuning
WIN = 256         # dst window width = segment-matmul N
TILE = 128        # edge slots per tile (= matmul K)
BANK = 512        # PSUM bank width in f32 cols
CHUNK_WINS = 6    # windows per PSUM chunk (6*256 = 1536 cols = 3 banks)
GXMAX = 8         # max tiles per dma_gather
SCH = 4           # src chunks (int16 gather indices => table <= 32767 rows)
NQ = 4            # SWDGE queues
SCRATCH = 16384   # dynamic DMA scratch bytes/partition
PREP = False       # prepare_only + trigger_dma (False: blocking dma_gather)
OFFS_PAD = 512.0  # one-hot match value for pad slots (never matches iota)


def _preprocess(src, dst, n_nodes, npc, cores):
    """Host-side edge partitioning (integer index metadata only)."""
    assert n_nodes % SCH == 0
    cn = n_nodes // SCH
    assert cn < 32768, "src-chunk must fit int16 gather indices"
    src = np.asarray(src).astype(np.int64)
    dst = np.asarray(dst).astype(np.int64)
    E = len(src)
    deg = np.bincount(dst, minlength=n_nodes)
    nrm = 1.0 / np.sqrt(np.maximum(deg, 1).astype(np.float64))

    core_of = dst // npc
    ldst = dst - core_of * npc
    win = ldst // WIN
    kch = src // cn
    n_wins = (npc + WIN - 1) // WIN
    n_regions = n_wins * SCH
    region = win * SCH + kch  # (w, k) region id within a core

    # program emission order of regions: psum-chunk major, then k, then w
    order_regions = []
    for p0 in range(0, n_wins, CHUNK_WINS):
        p1 = min(n_wins, p0 + CHUNK_WINS)
        for k in range(SCH):
            for w in range(p0, p1):
                order_regions.append(w * SCH + k)
    order_regions = np.array(order_regions, np.int64)
    region_pos = np.empty(n_regions, np.int64)
    region_pos[order_regions] = np.arange(n_regions)

    # per-core edge counts per region; shared tile schedule = per-region max
    cnt = np.zeros((cores, n_regions), np.int64)
    for c in range(cores):
        m = core_of == c
        cnt[c] = np.bincount(region[m], minlength=n_regions)
    tiles_r = -(-cnt.max(axis=0) // TILE)  # ceil
    # every window must write its PSUM cols at least once
    win_tiles = tiles_r.reshape(n_wins, SCH).sum(axis=1)
    for w in np.nonzero(win_tiles == 0)[0]:
        tiles_r[w * SCH] = 1

    T_sorted = tiles_r[order_regions]
    tile_base_sorted = np.zeros(n_regions + 1, np.int64)
    np.cumsum(T_sorted, out=tile_base_sorted[1:])
    n_tiles = int(tile_base_sorted[-1])
    n_slots = n_tiles * TILE
    slot_base_pos = tile_base_sorted[:-1] * TILE  # by emission pos

    # slot assignment: sort edges by (core, region emission pos, src)
    rpos = region_pos[region]
    glob = np.lexsort((src, rpos, core_of))
    cs, rs = core_of[glob], rpos[glob]
    runkey = cs * n_regions + rs
    starts = np.searchsorted(runkey, np.arange(cores * n_regions))
    rank = np.arange(E) - starts[runkey]
    slot = slot_base_pos[rs] + rank

    core_bounds = np.searchsorted(cs, np.arange(cores + 1))

    per_core = []
    for c in range(cores):
        s, e = core_bounds[c], core_bounds[c + 1]
        sl = slot[s:e]
        g = glob[s:e]
        gidx = np.zeros(n_slots, np.int16)        # pads gather row 0 of chunk
        offs = np.full(n_slots, OFFS_PAD, np.float32)
        nrmd = np.zeros(n_slots, np.float32)
        gidx[sl] = (src[g] - kch[g] * cn).astype(np.int16)
        offs[sl] = (ldst[g] - win[g] * WIN).astype(np.float32)
        nrmd[sl] = nrm[dst[g]].astype(np.float32)

        # [n_slots] -> [128, n_tiles]: slot j of tile t at [j, t]
        def t_(a, dt):
            return np.ascontiguousarray(a.reshape(n_tiles, TILE).T.astype(dt))

        # int16 idx wrap for dma_gather: within-instruction idx i at
        # [i % 16, i // 16], replicated across the 8 16-partition groups.
        # Instruction = run of whole tiles, so per-tile 8-col blocks suffice.
        a = gidx.reshape(n_tiles, 8, 16)          # [t, i//16, i%16]
        wrapped = np.transpose(a, (2, 0, 1)).reshape(16, n_tiles * 8)
        gidx16 = np.ascontiguousarray(np.tile(wrapped, (8, 1)))  # [128, 8*NT]

        per_core.append(dict(
            gidx16=gidx16,
            offs=t_(offs, np.float32),
            nrmd=t_(nrmd, np.float32),
        ))

    # gather batches: consecutive same-k regions packed up to GXMAX tiles,
    # never crossing a psum-chunk boundary
    groups = []  # (k, t_start, t_end)
    n_chunks = -(-n_wins // CHUNK_WINS)
    pos = 0
    t_acc = 0
    for p0 in range(0, n_wins, CHUNK_WINS):
        p1 = min(n_wins, p0 + CHUNK_WINS)
        for k in range(SCH):
            run = []  # tile counts of this (chunk, k) run of regions
            for w in range(p0, p1):
                run.append(int(tiles_r[w * SCH + k]))
            total = sum(run)
            # split [t_acc, t_acc+total) into <=GXMAX batches
            a = 0
            while a < total:
                b = min(total, a + GXMAX)
                groups.append((k, t_acc + a, t_acc + b))
                a = b
            t_acc += total
            pos += len(run)
    assert t_acc == n_tiles

    # window of each tile (for matmul column/bank mapping)
    win_of_tile = np.empty(n_tiles, np.int64)
    t = 0
    for p0 in range(0, n_wins, CHUNK_WINS):
        p1 = min(n_wins, p0 + CHUNK_WINS)
        for k in range(SCH):
            for w in range(p0, p1):
                nt = int(tiles_r[w * SCH + k])
                win_of_tile[t:t + nt] = w
                t += nt

    return dict(
        groups=groups,
        n_wins=n_wins,
        n_tiles=n_tiles,
        win_of_tile=win_of_tile,
        nrm=nrm,
        per_core=per_core,
    )


def _build_program(sched, n_nodes, npc, split_waits=True):
    """Build the single SPMD Bass/Tile program (identical for all cores)."""
    n_wins = sched["n_wins"]
    n_tiles = sched["n_tiles"]
    win_of_tile = sched["win_of_tile"]
    groups = sched["groups"]
    cn = n_nodes // SCH
    padn = n_wins * WIN            # padded local dst count (cols of out^T)
    n_chunks = -(-n_wins // CHUNK_WINS)

    nc = bacc.Bacc("TRN2", target_bir_lowering=False,
                   num_swdge_queues=NQ, dynamic_dma_scratch_size=SCRATCH)
    hb = nc.declare_dram_parameter("hb", [n_nodes, D], BF16, isOutput=False)
    hTd = nc.declare_dram_parameter("hTd", [D, padn], BF16, isOutput=False)
    gidx_p = nc.declare_dram_parameter("gidx16", [TILE, 8 * n_tiles], I16, isOutput=False)
    offs_p = nc.declare_dram_parameter("offs", [TILE, n_tiles], F32, isOutput=False)
    nrmd_p = nc.declare_dram_parameter("nrmd", [TILE, n_tiles], F32, isOutput=False)
    wt_p = nc.declare_dram_parameter("wt", [2 * D, HID], BF16, isOutput=False)
    bias_p = nc.declare_dram_parameter("bias_c", [HID, 1], F32, isOutput=False)
    out_p = nc.declare_dram_parameter("out", [HID, padn], F32, isOutput=True)

    # tiles grouped per psum chunk
    chunk_of_tile = win_of_tile // CHUNK_WINS

    with tile.TileContext(nc) as tc:
        with (
            tc.tile_pool(name="const", bufs=1) as const,
            tc.tile_pool(name="g", bufs=3) as gpool,
            tc.tile_pool(name="oh", bufs=3) as ohpool,
            tc.tile_pool(name="ht", bufs=2) as htpool,
            tc.tile_pool(name="at", bufs=2) as atpool,
            tc.tile_pool(name="y", bufs=6) as ypool,
            tc.tile_pool(name="aggps", bufs=1, space="PSUM") as agg_ps,
            tc.tile_pool(name="scrps", bufs=2, space="PSUM") as scr_ps,
        ):
            # ---- constants / metadata ----
            gidx_sb = const.tile([TILE, 8 * n_tiles], I16)
            nc.sync.dma_start(gidx_sb[:], gidx_p[:])
            offs_sb = const.tile([TILE, n_tiles], F32)
            nc.sync.dma_start(offs_sb[:], offs_p[:])
            nrmd_sb = const.tile([TILE, n_tiles], F32)
            nc.sync.dma_start(nrmd_sb[:], nrmd_p[:])

            w1_sb = const.tile([D, HID], BF16)
            nc.sync.dma_start(w1_sb[:], wt_p[0:D, :])
            w2_sb = const.tile([D, HID], BF16)
            nc.sync.dma_start(w2_sb[:], wt_p[D:2 * D, :])
            bias_sb = const.tile([HID, 1], F32)
            nc.sync.dma_start(bias_sb[:], bias_p[:])
            ones_sb = const.tile([128, 128], BF16)
            nc.vector.memset(ones_sb[:], 1.0)
            iota_i = const.tile([128, WIN], I32)
            nc.gpsimd.iota(iota_i[:], pattern=[[1, WIN]], base=0, channel_multiplier=0)
            iota_b = const.tile([128, WIN], BF16)
            nc.vector.tensor_copy(iota_b[:], iota_i[:])

            # shared num_idxs registers for dma_gather (one per distinct size)
            ni_regs = {}

            def ni_reg(n):
                if n not in ni_regs:
                    r = nc.gpsimd.alloc_register()
                    nc.gpsimd.reg_mov(r, n)
                    ni_regs[n] = r
                return ni_regs[n]

            dma_sems = [nc.alloc_semaphore(f"gdma{q}") for q in range(NQ)]

            # per-chunk batch lists
            batches_of_chunk = [[] for _ in range(n_chunks)]
            for gi, (k, ta, tb) in enumerate(groups):
                ch = int(chunk_of_tile[ta])
                assert int(chunk_of_tile[tb - 1]) == ch
                batches_of_chunk[ch].append((k, ta, tb))

            qrr = 0  # round-robin queue cursor

            # ---- main loop over dst chunks ----
            for ch in range(n_chunks):
                w0 = ch * CHUNK_WINS
                w1 = min(n_wins, w0 + CHUNK_WINS)
                cw = (w1 - w0) * WIN
                col0 = w0 * WIN

                tlist = [t for (k, ta, tb) in batches_of_chunk[ch]
                         for t in range(ta, tb)]
                # first/last program-order touch per psum bank in this chunk
                first_of_bank, last_of_bank = {}, {}
                for t in tlist:
                    bk = (int(win_of_tile[t]) - w0) * WIN // BANK
                    first_of_bank.setdefault(bk, t)
                    last_of_bank[bk] = t

                pagg = agg_ps.tile([128, CHUNK_WINS * WIN], F32, tag="pagg")

                for (k, ta, tb) in batches_of_chunk[ch]:
                    gt = tb - ta
                    G = gpool.tile([128, GXMAX, D], BF16, tag="G")
                    q = qrr % NQ
                    qrr += 1
                    if PREP:
                        nc.gpsimd.dma_gather(
                            out_ap=G[:, :gt, :],
                            in_ap=hb[k * cn:(k + 1) * cn, :],
                            idxs_ap=gidx_sb[:, 8 * ta:8 * tb],
                            num_idxs=TILE * gt,
                            num_idxs_reg=ni_reg(TILE * gt),
                            elem_size=D,
                            prepare_only=True,
                            sem=dma_sems[q],
                            queue_num=q,
                        )
                        nc.gpsimd.trigger_dma(count=None, queue_num=q)
                    else:
                        nc.gpsimd.dma_gather(
                            out_ap=G[:, :gt, :],
                            in_ap=hb[k * cn:(k + 1) * cn, :],
                            idxs_ap=gidx_sb[:, 8 * ta:8 * tb],
                            num_idxs=TILE * gt,
                            num_idxs_reg=ni_reg(TILE * gt),
                            elem_size=D,
                            queue_num=q,
                        )

                    oh = ohpool.tile([128, GXMAX, WIN], BF16, tag="oh")
                    for x in range(gt):
                        t = ta + x
                        nc.vector.tensor_scalar(
                            out=oh[:, x, :],
                            in0=iota_b[:],
                            scalar1=offs_sb[:, t:t + 1],
                            scalar2=nrmd_sb[:, t:t + 1],
                            op0=mybir.AluOpType.is_equal,
                            op1=mybir.AluOpType.mult,
                        )
                    for x in range(gt):
                        t = ta + x
                        col = (int(win_of_tile[t]) - w0) * WIN
                        bk = col // BANK
                        nc.tensor.matmul(
                            pagg[:, col:col + WIN],
                            lhsT=G[:, x, :],
                            rhs=oh[:, x, :],
                            start=(first_of_bank[bk] == t),
                            stop=(last_of_bank[bk] == t),
                            skip_group_check=True,
                        )

                # evacuate agg chunk (cast to bf16; norms folded into oh/hb)
                aggT = atpool.tile([128, CHUNK_WINS * WIN], BF16, tag="aggT")
                nc.vector.tensor_copy(aggT[:, :cw], pagg[:, :cw])

                # h^T slab (host-pretransposed)
                hT = htpool.tile([128, CHUNK_WINS * WIN], BF16, tag="hT")
                nc.sync.dma_start(hT[:, :cw], hTd[:, col0:col0 + cw])

                # out^T = W1.T @ h^T + W2.T @ agg^T ; +bias; L2 normalize
                for bs in range(0, cw, BANK):
                    bw = min(BANK, cw - bs)
                    po = scr_ps.tile([128, BANK], F32, tag="po")
                    nc.tensor.matmul(po[:, :bw], lhsT=w1_sb[:], rhs=hT[:, bs:bs + bw],
                                     start=True, stop=False)
                    nc.tensor.matmul(po[:, :bw], lhsT=w2_sb[:], rhs=aggT[:, bs:bs + bw],
                                     start=False, stop=True)
                    y = ypool.tile([128, BANK], F32, tag="y")
                    nc.scalar.activation(y[:, :bw], po[:, :bw],
                                         mybir.ActivationFunctionType.Identity,
                                         bias=bias_sb[:])
                    z = ypool.tile([128, BANK], BF16, tag="z")
                    nc.scalar.square(z[:, :bw], y[:, :bw])
                    pr = scr_ps.tile([128, BANK], F32, tag="pr")
                    nc.tensor.matmul(pr[:, :bw], lhsT=ones_sb[:], rhs=z[:, :bw],
                                     start=True, stop=True)
                    lg = ypool.tile([128, BANK], F32, tag="lg")
                    nc.scalar.activation(lg[:, :bw], pr[:, :bw],
                                         mybir.ActivationFunctionType.Ln)
                    rs = ypool.tile([128, BANK], F32, tag="rs")
                    nc.scalar.activation(rs[:, :bw], lg[:, :bw],
                                         mybir.ActivationFunctionType.Exp,
                                         scale=-0.5)
                    of = ypool.tile([128, BANK], F32, tag="of")
                    nc.vector.tensor_tensor(out=of[:, :bw], in0=y[:, :bw],
                                            in1=rs[:, :bw], op=mybir.AluOpType.mult)
                    nc.sync.dma_start(out_p[:, col0 + bs:col0 + bs + bw], of[:, :bw])

    nc.finalize()
    if split_waits:
        _split_excess_waits(nc)
    return nc


def _run(h, weight, bias, src, dst, n_nodes, npc, cores, trace=False):
    sched = _preprocess(src, dst, n_nodes, npc, cores)
    nc = _build_program(sched, n_nodes, npc)

    padn = sched["n_wins"] * WIN
    h = np.asarray(h, dtype=np.float32)
    # gather table pre-scaled by rsqrt(deg_src)
    hb = (h * sched["nrm"][:, None].astype(np.float32)).astype(ml_dtypes.bfloat16)
    wt = np.asarray(weight, dtype=np.float32).astype(ml_dtypes.bfloat16)
    bias_c = np.ascontiguousarray(np.asarray(bias, dtype=np.float32).reshape(HID, 1))

    in_maps = []
    for c in range(cores):
        pc = sched["per_core"][c]
        hTd = np.zeros((D, padn), dtype=ml_dtypes.bfloat16)
        hTd[:, :npc] = h[c * npc:(c + 1) * npc].T.astype(ml_dtypes.bfloat16)
        in_maps.append(dict(
            hb=hb, hTd=np.ascontiguousarray(hTd),
            gidx16=pc["gidx16"], offs=pc["offs"], nrmd=pc["nrmd"],
            wt=wt, bias_c=bias_c,
        ))

    res = run_bass_kernel_spmd(nc, in_maps, core_ids=list(range(cores)), trace=trace)
    out = np.empty((cores * npc, HID), dtype=np.float32)
    for c in range(cores):
        out[c * npc:(c + 1) * npc] = res.results[c]["out"][:, :npc].T
    return out, res


def kernel(h, weight, bias, src, dst):
    out, _ = _run(h, weight, bias, src, dst, N_NODES, N_NODES // CORES, CORES)
    return out
